# revision 13
# baseline (speedup 1.0000x reference)
"""Multi-head attention (B=2, S=2048, D=1024, H=16) on 8 Trainium2 NeuronCores.

Sharding: batch x head-group. Core c handles batch b = c // 4 and heads
4*(c%4) .. 4*(c%4)+3.  Each core computes its slice of the QKV projection,
full attention for its 4 heads, and a partial (row-sliced) output
projection; the host sums the 4 partials per batch and adds b_out.

Device kernel layout notes (per core):
  - All matmul operands are fp16 (PSUM accumulation is fp32); absmax-relative
    error vs the fp32 reference is ~3e-4.
  - x^T, Q^T, K^T are kept feature-on-partition so QK^T and PV need no
    on-chip transposes: scores are computed as S^T = (K_h^T)^T-contraction
    with s_k on PSUM partitions, softmax exp runs on the scalar engine with
    the key-padding-mask bias folded in, and the softmax denominator comes
    for free from a ones-column appended to V (PV matmul M=65).
  - 1/sqrt(d_head) is folded into Wq/bq on the host (exact, power of two).
"""

import os
import sys

import numpy as np

for _p in ("/root/.axon_site/_ro/trn_rl_repo", "/opt/trn_rl_repo"):
    if os.path.isdir(_p) and _p not in sys.path:
        sys.path.insert(0, _p)

import concourse.bass as bass  # noqa: E402
import concourse.tile as tile  # noqa: E402
from concourse import bacc, mybir  # noqa: E402

B, S, D = 2, 2048, 1024
H, DH = 16, 64
NCORES = 8
GROUP = NCORES // B          # 4 cores per batch
HPC = H // GROUP             # 4 heads per core
KSUB = D // 128              # 8 contraction subtiles of 128
MSUB = (HPC * DH) // 128     # 2 row-chunks of the per-core head slice (256)
SC = S // 128                # 16 chunks of 128 along sequence
QC = S // 512                # 4 query chunks of 512

FP16 = mybir.dt.float16
F32 = mybir.dt.float32
ADD = mybir.AluOpType.add
MULT = mybir.AluOpType.mult
EXP = mybir.ActivationFunctionType.Exp

MASK_NEG = -30000.0  # exp(s + MASK_NEG) underflows to exactly 0 in fp32


def build_program():
    """Build the SPMD Bass program (identical on all 8 cores)."""
    nc = bacc.Bacc("TRN2", debug=False, num_devices=NCORES)

    xT = nc.dram_tensor("xT", [128, KSUB * S], FP16, kind="ExternalInput").ap()
    wq = nc.dram_tensor("wq", [128, KSUB * 256], FP16, kind="ExternalInput").ap()
    wk = nc.dram_tensor("wk", [128, KSUB * 256], FP16, kind="ExternalInput").ap()
    wv = nc.dram_tensor("wv", [128, KSUB * 256], FP16, kind="ExternalInput").ap()
    wo = nc.dram_tensor("wo", [128, MSUB * 1024], FP16, kind="ExternalInput").ap()
    bq = nc.dram_tensor("bq", [128, MSUB], F32, kind="ExternalInput").ap()
    bk = nc.dram_tensor("bk", [128, MSUB], F32, kind="ExternalInput").ap()
    bv = nc.dram_tensor("bv", [1, 256], F32, kind="ExternalInput").ap()
    mb = nc.dram_tensor("mb", [128, SC], F32, kind="ExternalInput").ap()
    outp = nc.dram_tensor("outp", [S, D], F32, kind="ExternalOutput").ap()

    with tile.TileContext(nc) as tc:
        _body(tc, xT, wq, wk, wv, wo, bq, bk, bv, mb, outp)

    nc.compile()
    return nc


def _body(tc, xT, wq, wk, wv, wo, bq, bk, bv, mb, outp):
    nc = tc.nc
    from contextlib import ExitStack

    with ExitStack() as ctx:
        const = ctx.enter_context(tc.tile_pool(name="const", bufs=1))

        # ---- load inputs -------------------------------------------------
        # Emission order matters: the first Q-projection matmul needs only
        # wq and the first xT subtile, so those DMAs go first.
        xT_sb = const.tile([128, KSUB, S], FP16, name="xT_sb", tag="xT_sb")
        wq_sb = const.tile([128, KSUB, 256], FP16, name="wq_sb", tag="wq_sb")
        wk_sb = const.tile([128, KSUB, 256], FP16, name="wk_sb", tag="wk_sb")
        wv_sb = const.tile([128, KSUB, 256], FP16, name="wv_sb", tag="wv_sb")
        wo_sb = const.tile([128, MSUB, 1024], FP16, name="wo_sb", tag="wo_sb")
        nc.sync.dma_start(out=wq_sb[:, 0, :], in_=wq[:, 0:256])
        nc.sync.dma_start(out=xT_sb[:, 0, 0:1024], in_=xT[:, 0:1024])
        nc.sync.dma_start(out=xT_sb[:, 0, 1024:S], in_=xT[:, 1024:S])
        for k in range(1, KSUB):
            nc.sync.dma_start(out=wq_sb[:, k, :], in_=wq[:, k * 256:(k + 1) * 256])
        nc.sync.dma_start(out=wk_sb[:], in_=wk.rearrange("p (a b) -> p a b", a=KSUB))
        for k in range(1, KSUB):
            nc.sync.dma_start(out=xT_sb[:, k, :], in_=xT[:, k * S:(k + 1) * S])
        nc.sync.dma_start(out=wv_sb[:], in_=wv.rearrange("p (a b) -> p a b", a=KSUB))
        nc.sync.dma_start(out=wo_sb[:], in_=wo.rearrange("p (a b) -> p a b", a=MSUB))
        bq_sb = const.tile([128, MSUB], F32, name="bq_sb", tag="bq_sb")
        nc.sync.dma_start(out=bq_sb[:], in_=bq[:, :])
        bk_sb = const.tile([128, MSUB], F32, name="bk_sb", tag="bk_sb")
        nc.sync.dma_start(out=bk_sb[:], in_=bk[:, :])
        bv_bc = const.tile([128, 256], F32, name="bv_bc", tag="bv_bc")
        nc.sync.dma_start(out=bv_bc[:], in_=bv.to_broadcast([128, 256]))
        mb_sb = const.tile([128, SC], F32, name="mb_sb", tag="mb_sb")
        nc.sync.dma_start(out=mb_sb[:], in_=mb[:, :])

        # Parity-split Q^T/K^T: even heads live in partitions 0-63 (upper
        # half zeroed), odd heads in 64-127 (lower half zeroed), so the
        # score matmuls contract over a full K=128 (zeros contribute 0).
        # Full-row matmuls keep the PE activity monitor in the warm state.
        QT_e = const.tile([128, MSUB, S], FP16, name="QT_e", tag="QT_e")
        QT_o = const.tile([128, MSUB, S], FP16, name="QT_o", tag="QT_o")
        KT_e = const.tile([128, MSUB, S], FP16, name="KT_e", tag="KT_e")
        KT_o = const.tile([128, MSUB, S], FP16, name="KT_o", tag="KT_o")
        for t in (QT_e, KT_e):
            nc.vector.memset(t[64:128, :, :], 0.0)
        for t in (QT_o, KT_o):
            nc.vector.memset(t[0:64, :, :], 0.0)
        ones_sb = const.tile([128, DH], F32, name="ones_sb", tag="ones_sb")
        nc.vector.memset(ones_sb[:], 1.0)
        # V' per head: [s_k(part), s_k chunk, d_head + ones column]
        VP = [const.tile([128, SC, DH + 1], FP16, name=f"vp{h}", tag=f"vp{h}")
              for h in range(HPC)]
        attnT = const.tile([128, MSUB, S], FP16, name="attnT", tag="attnT")

        # ---- QKV projection ---------------------------------------------
        with tc.tile_pool(name="psA", bufs=8, space="PSUM") as psA:
            for wsb, bsb, dste, dsto, pname in (
                    (wq_sb, bq_sb, QT_e, QT_o, "q"),
                    (wk_sb, bk_sb, KT_e, KT_o, "k")):
                ps = [[psA.tile([128, 512], F32, name=f"ps{pname}{m}{n}", tag="ps")
                       for n in range(QC)] for m in range(MSUB)]
                for k in range(KSUB):
                    for m in range(MSUB):
                        lhsT = wsb[:, k, m * 128:(m + 1) * 128]
                        for n in range(QC):
                            nc.tensor.matmul(
                                ps[m][n][:], lhsT,
                                xT_sb[:, k, n * 512:(n + 1) * 512],
                                start=(k == 0), stop=(k == KSUB - 1))
                for m in range(MSUB):
                    for n in range(QC):
                        nc.vector.tensor_tensor(
                            out=dste[0:64, m, n * 512:(n + 1) * 512],
                            in0=ps[m][n][0:64, :],
                            in1=bsb[0:64, m:m + 1].to_broadcast([64, 512]),
                            op=ADD)
                        nc.vector.tensor_tensor(
                            out=dsto[64:128, m, n * 512:(n + 1) * 512],
                            in0=ps[m][n][64:128, :],
                            in1=bsb[64:128, m:m + 1].to_broadcast([64, 512]),
                            op=ADD)
            # V (natural layout), fanned out into the per-head V' tiles
            for mt in range(SC):
                psv = psA.tile([128, 256], F32, name=f"psv{mt}", tag="ps")
                for k in range(KSUB):
                    nc.tensor.matmul(
                        psv[:], xT_sb[:, k, mt * 128:(mt + 1) * 128],
                        wv_sb[:, k, :],
                        start=(k == 0), stop=(k == KSUB - 1))
                for h in range(HPC):
                    nc.vector.tensor_tensor(
                        out=VP[h][:, mt, 0:DH],
                        in0=psv[:, h * DH:(h + 1) * DH],
                        in1=bv_bc[:, h * DH:(h + 1) * DH],
                        op=ADD)
            for h in range(HPC):
                nc.vector.memset(VP[h][:, :, DH:DH + 1], 1.0)

        # ---- attention (4 heads) ----------------------------------------
        # Global software-pipelined stream over (head, kc): PV matmuls lag
        # the S^T/exp stream by LAG chunks so the PE always has fill work
        # while ACT drains exps (keeps the PE HAM-warm at 2.4 GHz).
        LAG = 4
        with tc.tile_pool(name="psS", bufs=2, space="PSUM") as psS, \
             tc.tile_pool(name="psO", bufs=4, space="PSUM") as psO, \
             tc.tile_pool(name="ptp", bufs=LAG + 3) as ptp, \
             tc.tile_pool(name="dsc", bufs=1, space="DRAM") as dsc, \
             tc.tile_pool(name="misc", bufs=4) as misc:

            pso_of = {}
            pt_of = {}

            def emit_s(h, kc):
                ksub = h // 2
                KT_p = KT_e if h % 2 == 0 else KT_o
                QT_p = QT_e if h % 2 == 0 else QT_o
                pt = ptp.tile([128, S], FP16, name=f"pt{h}_{kc}", tag="pt")
                pt_of[(h, kc)] = pt
                for qp in range(QC // 2):
                    ss = psS.tile([128, 1024], F32, name=f"ss{h}_{kc}_{qp}",
                                  tag="ss")
                    for j in range(2):
                        qc = qp * 2 + j
                        nc.tensor.matmul(
                            ss[:, j * 512:(j + 1) * 512],
                            KT_p[:, ksub, kc * 128:(kc + 1) * 128],
                            QT_p[:, ksub, qc * 512:(qc + 1) * 512],
                            start=True, stop=True)
                    nc.scalar.activation(
                        out=pt[:, qp * 1024:(qp + 1) * 1024],
                        in_=ss[:],
                        func=EXP,
                        bias=mb_sb[:, kc:kc + 1],
                        scale=1.0)

            def emit_pv(h, kc):
                pso = pso_of[h]
                pt = pt_of.pop((h, kc))
                for qc in range(QC):
                    nc.tensor.matmul(
                        pso[qc][0:DH + 1, :], VP[h][:, kc, :],
                        pt[:, qc * 512:(qc + 1) * 512],
                        start=(kc == 0), stop=(kc == SC - 1))

            def emit_norm(h):
                # Copy O+denominator off PSUM (frees the O banks for the
                # next head). The denominators of the whole head are then
                # gathered via DRAM into a [128, 16] layout so a single
                # 128-lane reciprocal covers them, and the reciprocals are
                # broadcast back across partitions with a stride-0-source
                # DMA from DRAM. Everything here is DVE+DMA only — the
                # matmul stream never stalls on it.
                qpart = (h % 2) * 64
                ksub = h // 2
                pso = pso_of.pop(h)
                if h == HPC - 1:
                    # last head: the output projection is waiting on this,
                    # so use a short-latency path (serial reciprocal + DMA
                    # broadcast through DRAM) that never touches the PE.
                    rd = dsc.tile([QC, 512], F32, name=f"rdf{h}", tag=f"rdf{h}")
                    for qc in range(QC):
                        rec = misc.tile([DH + 1, 512], F32,
                                        name=f"rec{h}_{qc}", tag="rec")
                        nc.vector.reciprocal(out=rec[DH:DH + 1, :],
                                             in_=pso[qc][DH:DH + 1, :])
                        nc.sync.dma_start(out=rd[qc:qc + 1, :],
                                          in_=rec[DH:DH + 1, :])
                        rbc = misc.tile([DH, 512], F32, name=f"rbc{h}_{qc}",
                                        tag="rbc")
                        nc.sync.dma_start(out=rbc[:],
                                          in_=rd[qc:qc + 1, :].to_broadcast(
                                              [DH, 512]))
                        on = misc.tile([DH, 512], FP16, name=f"on{h}_{qc}",
                                       tag="on")
                        nc.vector.tensor_tensor(out=on[:],
                                                in0=pso[qc][0:DH, :],
                                                in1=rbc[:], op=MULT)
                        nc.sync.dma_start(
                            out=attnT[qpart:qpart + 64, ksub,
                                      qc * 512:(qc + 1) * 512],
                            in_=on[:])
                    return
                dd = dsc.tile([QC, 512], F32, name=f"dd{h}", tag=f"dd{h}")
                rd = dsc.tile([QC, 512], F32, name=f"rd{h}", tag=f"rd{h}")
                ouns = []
                for qc in range(QC):
                    oun = misc.tile([DH + 1, 512], F32, name=f"oun{h}_{qc}",
                                    tag="oun")
                    nc.vector.tensor_copy(out=oun[:], in_=pso[qc][0:DH + 1, :])
                    nc.sync.dma_start(out=dd[qc:qc + 1, :],
                                      in_=oun[DH:DH + 1, :])
                    ouns.append(oun)
                dsb = misc.tile([128, QC, 4], F32, name=f"dsb{h}", tag="dsb")
                nc.sync.dma_start(
                    out=dsb[:],
                    in_=dd.rearrange("a (p b) -> p a b", p=128))
                rsb = misc.tile([128, QC, 4], F32, name=f"rsb{h}", tag="rsb")
                nc.vector.reciprocal(out=rsb[:], in_=dsb[:])
                for qc in range(QC):
                    nc.sync.dma_start(
                        out=rd[qc, :].rearrange("(p b) -> p b", p=128),
                        in_=rsb[:, qc, :])
                for qc in range(QC):
                    rbc = misc.tile([DH, 512], F32, name=f"rbc{h}_{qc}",
                                    tag="rbc")
                    nc.sync.dma_start(out=rbc[:],
                                      in_=rd[qc:qc + 1, :].to_broadcast(
                                          [DH, 512]))
                    on = misc.tile([DH, 512], FP16, name=f"on{h}_{qc}",
                                   tag="on")
                    nc.vector.tensor_tensor(out=on[:], in0=ouns[qc][0:DH, :],
                                            in1=rbc[:], op=MULT)
                    # partition-offset move into the concat layout
                    nc.sync.dma_start(
                        out=attnT[qpart:qpart + 64, ksub,
                                  qc * 512:(qc + 1) * 512],
                        in_=on[:])

            items = [(h, kc) for h in range(HPC) for kc in range(SC)]
            for i, (h, kc) in enumerate(items):
                if kc == 0:
                    pso_of[h] = [psO.tile([128, 512], F32,
                                          name=f"pso{h}_{qc}", tag="pso")
                                 for qc in range(QC)]
                emit_s(h, kc)
                if i >= LAG:
                    hh, kk = items[i - LAG]
                    emit_pv(hh, kk)
                    if kk == SC - 1:
                        emit_norm(hh)
            for j in range(len(items) - LAG, len(items)):
                hh, kk = items[j]
                emit_pv(hh, kk)
                if kk == SC - 1:
                    emit_norm(hh)

        # ---- output projection (row slice of W_out) ---------------------
        with tc.tile_pool(name="psB", bufs=4, space="PSUM") as psB, \
             tc.tile_pool(name="osb", bufs=3) as osb:
            # Warm-keeper matmuls: the last head's softmax normalization has
            # a ~10us DVE+DMA latency chain during which the PE would
            # otherwise idle long enough for the HAM to re-throttle the
            # clock. Chew on resident constants to keep it warm.
            warm = psB.tile([128, 512], F32, name="warm", tag="warm")
            for _ in range(40):
                nc.tensor.matmul(warm[:], wo_sb[:, 0, 0:128],
                                 wo_sb[:, 0, 0:512], start=True, stop=True)
            for mt in range(SC):
                pso2 = [psB.tile([128, 512], F32, name=f"po{mt}{n}", tag="po")
                        for n in range(2)]
                for k in range(MSUB):
                    lhsT = attnT[:, k, mt * 128:(mt + 1) * 128]
                    for n in range(2):
                        nc.tensor.matmul(
                            pso2[n][:], lhsT,
                            wo_sb[:, k, n * 512:(n + 1) * 512],
                            start=(k == 0), stop=(k == MSUB - 1))
                ot = osb.tile([128, 1024], F32, name=f"ot{mt}", tag="ot")
                for n in range(2):
                    nc.vector.tensor_copy(out=ot[:, n * 512:(n + 1) * 512],
                                          in_=pso2[n][:])
                nc.sync.dma_start(out=outp[mt * 128:(mt + 1) * 128, :],
                                  in_=ot[:])


# --------------------------------------------------------------------------
# host side
# --------------------------------------------------------------------------

_PROGRAM = None
LAST_RESULT = None
TRACE = False


def _get_program():
    global _PROGRAM
    if _PROGRAM is None:
        nc = build_program()
        from concourse.bass_interp import get_hw_module
        nc.m = get_hw_module(nc.m)
        _PROGRAM = nc
    return _PROGRAM


def _ksub_layout(a, inner):
    """[K*128, F] -> [128, K*F] with row index k*128+p -> (p, k)."""
    k = a.shape[0] // 128
    return np.ascontiguousarray(
        a.reshape(k, 128, inner).transpose(1, 0, 2).reshape(128, k * inner))


def make_in_maps(x, mask, W_qkv, b_qkv, W_out, b_out):
    x = np.asarray(x, np.float32)
    mask = np.asarray(mask)
    W_qkv = np.asarray(W_qkv, np.float32)
    b_qkv = np.asarray(b_qkv, np.float32)
    W_out = np.asarray(W_out, np.float32)

    in_maps = []
    for c in range(NCORES):
        b = c // GROUP
        hg = c % GROUP
        cs = hg * 256
        xT_r = _ksub_layout(np.ascontiguousarray(x[b].T), S).astype(np.float16)
        wq_r = _ksub_layout(W_qkv[:, cs:cs + 256] * 0.125, 256).astype(np.float16)
        wk_r = _ksub_layout(W_qkv[:, D + cs:D + cs + 256], 256).astype(np.float16)
        wv_r = _ksub_layout(W_qkv[:, 2 * D + cs:2 * D + cs + 256], 256).astype(np.float16)
        wo_r = _ksub_layout(W_out[cs:cs + 256, :], 1024).astype(np.float16)
        bq_r = np.ascontiguousarray(
            (b_qkv[cs:cs + 256] * 0.125).reshape(MSUB, 128).T).astype(np.float32)
        bk_r = np.ascontiguousarray(
            b_qkv[D + cs:D + cs + 256].reshape(MSUB, 128).T).astype(np.float32)
        bv_r = b_qkv[2 * D + cs:2 * D + cs + 256].reshape(1, 256).astype(np.float32)
        mb_r = np.ascontiguousarray(
            np.where(mask[b], 0.0, MASK_NEG).astype(np.float32)
            .reshape(SC, 128).T)
        in_maps.append({
            "xT": xT_r, "wq": wq_r, "wk": wk_r, "wv": wv_r, "wo": wo_r,
            "bq": bq_r, "bk": bk_r, "bv": bv_r, "mb": mb_r,
        })
    return in_maps


def kernel(x, mask, W_qkv, b_qkv, W_out, b_out):
    global LAST_RESULT
    from concourse import bass_utils

    nc = _get_program()
    in_maps = make_in_maps(x, mask, W_qkv, b_qkv, W_out, b_out)
    br = bass_utils.run_bass_kernel_spmd(
        nc, in_maps, core_ids=list(range(NCORES)), trace=TRACE)
    LAST_RESULT = br

    b_out = np.asarray(b_out, np.float32)
    out = np.zeros((B, S, D), np.float32)
    for c in range(NCORES):
        out[c // GROUP] += br.results[c]["outp"]
    out += b_out
    return out


# revision 14
# speedup vs baseline: 1.0941x; 1.0941x over previous
"""Multi-head attention (B=2, S=2048, D=1024, H=16) on 8 Trainium2 NeuronCores.

Sharding: batch x head-group. Core c handles batch b = c // 4 and heads
4*(c%4) .. 4*(c%4)+3.  Each core computes its slice of the QKV projection,
full attention for its 4 heads, and a partial (row-sliced) output
projection; the host sums the 4 partials per batch and adds b_out.

Device kernel layout notes (per core):
  - All matmul operands are fp16 (PSUM accumulation is fp32); absmax-relative
    error vs the fp32 reference is ~3e-4.
  - x^T, Q^T, K^T are kept feature-on-partition so QK^T and PV need no
    on-chip transposes: scores are computed as S^T = (K_h^T)^T-contraction
    with s_k on PSUM partitions, softmax exp runs on the scalar engine with
    the key-padding-mask bias folded in, and the softmax denominator comes
    for free from a ones-column appended to V (PV matmul M=65).
  - 1/sqrt(d_head) is folded into Wq/bq on the host (exact, power of two).
"""

import os
import sys

import numpy as np

for _p in ("/root/.axon_site/_ro/trn_rl_repo", "/opt/trn_rl_repo"):
    if os.path.isdir(_p) and _p not in sys.path:
        sys.path.insert(0, _p)

import concourse.bass as bass  # noqa: E402
import concourse.tile as tile  # noqa: E402
from concourse import bacc, mybir  # noqa: E402

B, S, D = 2, 2048, 1024
H, DH = 16, 64
NCORES = 8
GROUP = NCORES // B          # 4 cores per batch
HPC = H // GROUP             # 4 heads per core
KSUB = D // 128              # 8 contraction subtiles of 128
MSUB = (HPC * DH) // 128     # 2 row-chunks of the per-core head slice (256)
SC = S // 128                # 16 chunks of 128 along sequence
QC = S // 512                # 4 query chunks of 512

FP16 = mybir.dt.float16
F32 = mybir.dt.float32
ADD = mybir.AluOpType.add
MULT = mybir.AluOpType.mult
EXP = mybir.ActivationFunctionType.Exp

MASK_NEG = -30000.0  # exp(s + MASK_NEG) underflows to exactly 0 in fp32


def build_program():
    """Build the SPMD Bass program (identical on all 8 cores)."""
    nc = bacc.Bacc("TRN2", debug=False, num_devices=NCORES)

    xT = nc.dram_tensor("xT", [128, KSUB * S], FP16, kind="ExternalInput").ap()
    wq = nc.dram_tensor("wq", [128, KSUB * 256], FP16, kind="ExternalInput").ap()
    wk = nc.dram_tensor("wk", [128, KSUB * 256], FP16, kind="ExternalInput").ap()
    wv = nc.dram_tensor("wv", [128, KSUB * 256], FP16, kind="ExternalInput").ap()
    wo = nc.dram_tensor("wo", [128, MSUB * 1024], FP16, kind="ExternalInput").ap()
    bq = nc.dram_tensor("bq", [128, MSUB], F32, kind="ExternalInput").ap()
    bk = nc.dram_tensor("bk", [128, MSUB], F32, kind="ExternalInput").ap()
    bv = nc.dram_tensor("bv", [1, 256], F32, kind="ExternalInput").ap()
    mb = nc.dram_tensor("mb", [128, SC], F32, kind="ExternalInput").ap()
    outp = nc.dram_tensor("outp", [S, D], F32, kind="ExternalOutput").ap()

    with tile.TileContext(nc) as tc:
        _body(tc, xT, wq, wk, wv, wo, bq, bk, bv, mb, outp)

    nc.compile()
    return nc


def _body(tc, xT, wq, wk, wv, wo, bq, bk, bv, mb, outp):
    nc = tc.nc
    from contextlib import ExitStack

    with ExitStack() as ctx:
        const = ctx.enter_context(tc.tile_pool(name="const", bufs=1))

        # ---- load inputs -------------------------------------------------
        # Emission order matters: the first Q-projection matmul needs only
        # wq and the first xT subtile, so those DMAs go first.
        xT_sb = const.tile([128, KSUB, S], FP16, name="xT_sb", tag="xT_sb")
        wq_sb = const.tile([128, KSUB, 256], FP16, name="wq_sb", tag="wq_sb")
        wk_sb = const.tile([128, KSUB, 256], FP16, name="wk_sb", tag="wk_sb")
        wv_sb = const.tile([128, KSUB, 256], FP16, name="wv_sb", tag="wv_sb")
        wo_sb = const.tile([128, MSUB, 1024], FP16, name="wo_sb", tag="wo_sb")
        nc.sync.dma_start(out=wq_sb[:], in_=wq.rearrange("p (a b) -> p a b", a=KSUB))
        nc.sync.dma_start(out=xT_sb[:, 0, :], in_=xT[:, 0:S])
        nc.sync.dma_start(out=wk_sb[:], in_=wk.rearrange("p (a b) -> p a b", a=KSUB))
        for k in range(1, KSUB):
            nc.sync.dma_start(out=xT_sb[:, k, :], in_=xT[:, k * S:(k + 1) * S])
        nc.sync.dma_start(out=wv_sb[:], in_=wv.rearrange("p (a b) -> p a b", a=KSUB))
        nc.sync.dma_start(out=wo_sb[:], in_=wo.rearrange("p (a b) -> p a b", a=MSUB))
        bq_sb = const.tile([128, MSUB], F32, name="bq_sb", tag="bq_sb")
        nc.sync.dma_start(out=bq_sb[:], in_=bq[:, :])
        bk_sb = const.tile([128, MSUB], F32, name="bk_sb", tag="bk_sb")
        nc.sync.dma_start(out=bk_sb[:], in_=bk[:, :])
        bv_bc = const.tile([128, 256], F32, name="bv_bc", tag="bv_bc")
        nc.sync.dma_start(out=bv_bc[:], in_=bv.to_broadcast([128, 256]))
        mb_sb = const.tile([128, SC], F32, name="mb_sb", tag="mb_sb")
        nc.sync.dma_start(out=mb_sb[:], in_=mb[:, :])

        # Parity-split Q^T/K^T: even heads live in partitions 0-63 (upper
        # half zeroed), odd heads in 64-127 (lower half zeroed), so the
        # score matmuls contract over a full K=128 (zeros contribute 0).
        # Full-row matmuls keep the PE activity monitor in the warm state.
        QT_e = const.tile([128, MSUB, S], FP16, name="QT_e", tag="QT_e")
        QT_o = const.tile([128, MSUB, S], FP16, name="QT_o", tag="QT_o")
        KT_e = const.tile([128, MSUB, S], FP16, name="KT_e", tag="KT_e")
        KT_o = const.tile([128, MSUB, S], FP16, name="KT_o", tag="KT_o")
        for t in (QT_e, KT_e):
            nc.vector.memset(t[64:128, :, :], 0.0)
        for t in (QT_o, KT_o):
            nc.vector.memset(t[0:64, :, :], 0.0)
        ones_sb = const.tile([128, DH], F32, name="ones_sb", tag="ones_sb")
        nc.vector.memset(ones_sb[:], 1.0)
        # V' per head: [s_k(part), s_k chunk, d_head + ones column]
        VP = [const.tile([128, SC, DH + 1], FP16, name=f"vp{h}", tag=f"vp{h}")
              for h in range(HPC)]
        attnT = const.tile([128, MSUB, S], FP16, name="attnT", tag="attnT")

        # ---- QKV projection ---------------------------------------------
        with tc.tile_pool(name="psA", bufs=8, space="PSUM") as psA:
            for wsb, bsb, dste, dsto, pname in (
                    (wq_sb, bq_sb, QT_e, QT_o, "q"),
                    (wk_sb, bk_sb, KT_e, KT_o, "k")):
                ps = [[psA.tile([128, 512], F32, name=f"ps{pname}{m}{n}", tag="ps")
                       for n in range(QC)] for m in range(MSUB)]
                for k in range(KSUB):
                    for m in range(MSUB):
                        lhsT = wsb[:, k, m * 128:(m + 1) * 128]
                        for n in range(QC):
                            nc.tensor.matmul(
                                ps[m][n][:], lhsT,
                                xT_sb[:, k, n * 512:(n + 1) * 512],
                                start=(k == 0), stop=(k == KSUB - 1))
                for m in range(MSUB):
                    for n in range(QC):
                        nc.vector.tensor_tensor(
                            out=dste[0:64, m, n * 512:(n + 1) * 512],
                            in0=ps[m][n][0:64, :],
                            in1=bsb[0:64, m:m + 1].to_broadcast([64, 512]),
                            op=ADD)
                        nc.vector.tensor_tensor(
                            out=dsto[64:128, m, n * 512:(n + 1) * 512],
                            in0=ps[m][n][64:128, :],
                            in1=bsb[64:128, m:m + 1].to_broadcast([64, 512]),
                            op=ADD)
            # V (natural layout), fanned out into the per-head V' tiles
            for mt in range(SC):
                psv = psA.tile([128, 256], F32, name=f"psv{mt}", tag="ps")
                for k in range(KSUB):
                    nc.tensor.matmul(
                        psv[:], xT_sb[:, k, mt * 128:(mt + 1) * 128],
                        wv_sb[:, k, :],
                        start=(k == 0), stop=(k == KSUB - 1))
                for h in range(HPC):
                    nc.vector.tensor_tensor(
                        out=VP[h][:, mt, 0:DH],
                        in0=psv[:, h * DH:(h + 1) * DH],
                        in1=bv_bc[:, h * DH:(h + 1) * DH],
                        op=ADD)
            for h in range(HPC):
                nc.vector.memset(VP[h][:, :, DH:DH + 1], 1.0)

        # ---- attention (4 heads) ----------------------------------------
        # Global software-pipelined stream over (head, kc): PV matmuls lag
        # the S^T/exp stream by LAG chunks so the PE always has fill work
        # while ACT drains exps (keeps the PE HAM-warm at 2.4 GHz).
        LAG = 4
        with tc.tile_pool(name="psS", bufs=2, space="PSUM") as psS, \
             tc.tile_pool(name="psO", bufs=4, space="PSUM") as psO, \
             tc.tile_pool(name="ptp", bufs=LAG + 3) as ptp, \
             tc.tile_pool(name="dsc", bufs=1, space="DRAM") as dsc, \
             tc.tile_pool(name="misc", bufs=4) as misc:

            pso_of = {}
            pt_of = {}

            def emit_s(h, kc):
                ksub = h // 2
                KT_p = KT_e if h % 2 == 0 else KT_o
                QT_p = QT_e if h % 2 == 0 else QT_o
                pt = ptp.tile([128, S], FP16, name=f"pt{h}_{kc}", tag="pt")
                pt_of[(h, kc)] = pt
                for qp in range(QC // 2):
                    ss = psS.tile([128, 1024], F32, name=f"ss{h}_{kc}_{qp}",
                                  tag="ss")
                    for j in range(2):
                        qc = qp * 2 + j
                        nc.tensor.matmul(
                            ss[:, j * 512:(j + 1) * 512],
                            KT_p[:, ksub, kc * 128:(kc + 1) * 128],
                            QT_p[:, ksub, qc * 512:(qc + 1) * 512],
                            start=True, stop=True)
                    nc.scalar.activation(
                        out=pt[:, qp * 1024:(qp + 1) * 1024],
                        in_=ss[:],
                        func=EXP,
                        bias=mb_sb[:, kc:kc + 1],
                        scale=1.0)

            def emit_pv(h, kc):
                pso = pso_of[h]
                pt = pt_of.pop((h, kc))
                for qc in range(QC):
                    nc.tensor.matmul(
                        pso[qc][0:DH + 1, :], VP[h][:, kc, :],
                        pt[:, qc * 512:(qc + 1) * 512],
                        start=(kc == 0), stop=(kc == SC - 1))

            def emit_norm(h):
                # Copy O+denominator off PSUM (frees the O banks for the
                # next head). The denominators of the whole head are then
                # gathered via DRAM into a [128, 16] layout so a single
                # 128-lane reciprocal covers them, and the reciprocals are
                # broadcast back across partitions with a stride-0-source
                # DMA from DRAM. Everything here is DVE+DMA only — the
                # matmul stream never stalls on it.
                qpart = (h % 2) * 64
                ksub = h // 2
                pso = pso_of.pop(h)
                dd = dsc.tile([QC, 512], F32, name=f"dd{h}", tag=f"dd{h}")
                rd = dsc.tile([QC, 512], F32, name=f"rd{h}", tag=f"rd{h}")
                ouns = []
                for qc in range(QC):
                    oun = misc.tile([DH + 1, 512], F32, name=f"oun{h}_{qc}",
                                    tag="oun")
                    nc.vector.tensor_copy(out=oun[:], in_=pso[qc][0:DH + 1, :])
                    nc.sync.dma_start(out=dd[qc:qc + 1, :],
                                      in_=oun[DH:DH + 1, :])
                    ouns.append(oun)
                dsb = misc.tile([128, QC, 4], F32, name=f"dsb{h}", tag="dsb")
                nc.sync.dma_start(
                    out=dsb[:],
                    in_=dd.rearrange("a (p b) -> p a b", p=128))
                rsb = misc.tile([128, QC, 4], F32, name=f"rsb{h}", tag="rsb")
                nc.vector.reciprocal(out=rsb[:], in_=dsb[:])
                for qc in range(QC):
                    nc.sync.dma_start(
                        out=rd[qc, :].rearrange("(p b) -> p b", p=128),
                        in_=rsb[:, qc, :])
                for qc in range(QC):
                    rbc = misc.tile([DH, 512], F32, name=f"rbc{h}_{qc}",
                                    tag="rbc")
                    nc.sync.dma_start(out=rbc[:],
                                      in_=rd[qc:qc + 1, :].to_broadcast(
                                          [DH, 512]))
                    on = misc.tile([DH, 512], FP16, name=f"on{h}_{qc}",
                                   tag="on")
                    nc.vector.tensor_tensor(out=on[:], in0=ouns[qc][0:DH, :],
                                            in1=rbc[:], op=MULT)
                    # partition-offset move into the concat layout
                    nc.sync.dma_start(
                        out=attnT[qpart:qpart + 64, ksub,
                                  qc * 512:(qc + 1) * 512],
                        in_=on[:])

            items = [(h, kc) for h in range(HPC) for kc in range(SC)]
            for i, (h, kc) in enumerate(items):
                if kc == 0:
                    pso_of[h] = [psO.tile([128, 512], F32,
                                          name=f"pso{h}_{qc}", tag="pso")
                                 for qc in range(QC)]
                emit_s(h, kc)
                if i >= LAG:
                    hh, kk = items[i - LAG]
                    emit_pv(hh, kk)
                    if kk == SC - 1:
                        emit_norm(hh)
            for j in range(len(items) - LAG, len(items)):
                hh, kk = items[j]
                emit_pv(hh, kk)
                if kk == SC - 1:
                    emit_norm(hh)

        # ---- output projection (row slice of W_out) ---------------------
        with tc.tile_pool(name="psB", bufs=4, space="PSUM") as psB, \
             tc.tile_pool(name="osb", bufs=3) as osb:
            # Warm-keeper matmuls: the last head's softmax normalization has
            # a ~10us DVE+DMA latency chain during which the PE would
            # otherwise idle long enough for the HAM to re-throttle the
            # clock. Chew on resident constants to keep it warm.
            warm = psB.tile([128, 512], F32, name="warm", tag="warm")
            for _ in range(55):
                nc.tensor.matmul(warm[:], wo_sb[:, 0, 0:128],
                                 wo_sb[:, 0, 0:512], start=True, stop=True)
            for mt in range(SC):
                pso2 = [psB.tile([128, 512], F32, name=f"po{mt}{n}", tag="po")
                        for n in range(2)]
                for k in range(MSUB):
                    lhsT = attnT[:, k, mt * 128:(mt + 1) * 128]
                    for n in range(2):
                        nc.tensor.matmul(
                            pso2[n][:], lhsT,
                            wo_sb[:, k, n * 512:(n + 1) * 512],
                            start=(k == 0), stop=(k == MSUB - 1))
                ot = osb.tile([128, 1024], F32, name=f"ot{mt}", tag="ot")
                for n in range(2):
                    nc.vector.tensor_copy(out=ot[:, n * 512:(n + 1) * 512],
                                          in_=pso2[n][:])
                nc.sync.dma_start(out=outp[mt * 128:(mt + 1) * 128, :],
                                  in_=ot[:])


# --------------------------------------------------------------------------
# host side
# --------------------------------------------------------------------------

_PROGRAM = None
LAST_RESULT = None
TRACE = False


def _get_program():
    global _PROGRAM
    if _PROGRAM is None:
        nc = build_program()
        from concourse.bass_interp import get_hw_module
        nc.m = get_hw_module(nc.m)
        _PROGRAM = nc
    return _PROGRAM


def _ksub_layout(a, inner):
    """[K*128, F] -> [128, K*F] with row index k*128+p -> (p, k)."""
    k = a.shape[0] // 128
    return np.ascontiguousarray(
        a.reshape(k, 128, inner).transpose(1, 0, 2).reshape(128, k * inner))


def make_in_maps(x, mask, W_qkv, b_qkv, W_out, b_out):
    x = np.asarray(x, np.float32)
    mask = np.asarray(mask)
    W_qkv = np.asarray(W_qkv, np.float32)
    b_qkv = np.asarray(b_qkv, np.float32)
    W_out = np.asarray(W_out, np.float32)

    in_maps = []
    for c in range(NCORES):
        b = c // GROUP
        hg = c % GROUP
        cs = hg * 256
        xT_r = _ksub_layout(np.ascontiguousarray(x[b].T), S).astype(np.float16)
        wq_r = _ksub_layout(W_qkv[:, cs:cs + 256] * 0.125, 256).astype(np.float16)
        wk_r = _ksub_layout(W_qkv[:, D + cs:D + cs + 256], 256).astype(np.float16)
        wv_r = _ksub_layout(W_qkv[:, 2 * D + cs:2 * D + cs + 256], 256).astype(np.float16)
        wo_r = _ksub_layout(W_out[cs:cs + 256, :], 1024).astype(np.float16)
        bq_r = np.ascontiguousarray(
            (b_qkv[cs:cs + 256] * 0.125).reshape(MSUB, 128).T).astype(np.float32)
        bk_r = np.ascontiguousarray(
            b_qkv[D + cs:D + cs + 256].reshape(MSUB, 128).T).astype(np.float32)
        bv_r = b_qkv[2 * D + cs:2 * D + cs + 256].reshape(1, 256).astype(np.float32)
        mb_r = np.ascontiguousarray(
            np.where(mask[b], 0.0, MASK_NEG).astype(np.float32)
            .reshape(SC, 128).T)
        in_maps.append({
            "xT": xT_r, "wq": wq_r, "wk": wk_r, "wv": wv_r, "wo": wo_r,
            "bq": bq_r, "bk": bk_r, "bv": bv_r, "mb": mb_r,
        })
    return in_maps


def kernel(x, mask, W_qkv, b_qkv, W_out, b_out):
    global LAST_RESULT
    from concourse import bass_utils

    nc = _get_program()
    in_maps = make_in_maps(x, mask, W_qkv, b_qkv, W_out, b_out)
    br = bass_utils.run_bass_kernel_spmd(
        nc, in_maps, core_ids=list(range(NCORES)), trace=TRACE)
    LAST_RESULT = br

    b_out = np.asarray(b_out, np.float32)
    out = np.zeros((B, S, D), np.float32)
    for c in range(NCORES):
        out[c // GROUP] += br.results[c]["outp"]
    out += b_out
    return out
